# revision 44
# baseline (speedup 1.0000x reference)
"""Causal self-attention on 8 TRN2 NeuronCores.

Sharding: 4-way data parallel over batch x 2-way tensor parallel over heads.
Core c handles batch b=c//2, head group g=c%2 (heads 8g..8g+8).

Per-core device kernel (all matmuls bf16, fp32 PSUM accumulation):
  1. QKV projection from host-pretransposed xT [C, T]:
     - qT/kT produced head-dim-on-partitions ([128, T] tiles, head pairs)
     - V produced natural [T, 64/head] with an appended ones column (V')
     QKV pieces for chunk c+1 are interleaved into attention chunk c's
     pair gaps so PE fills ACT-bound attention slack.
  2. Causal attention per head pair, query-chunk-major (512-wide chunks).
     Per (pair, chunk): software-pipelined j-loop over k-blocks:
       S^T_j[k,q] for heads (2p, 2p+1) lands in ONE [128,1024] PSUM tile
       (h0 cols 0:512 = bank 0, h1 cols 512:1024 = bank 1); the two
       matmuls use 64-row PE tiles T0/T8 and run concurrently on HW.
       ONE exp on ACT per j (3D AP over both heads, scale=1/8 folded) ->
       es_j bf16; ONE diag-mask mul on DVE per diag j (3D AP, mask pair).
       Y'[65, 512] += V'_j.T @ es_j per head, emitted with a TWO-j lag so
       PE stays ahead of exp deps; diagonal k-blocks are processed FIRST
       so the loop ends on unmasked full-width blocks (short pair-end
       join). Y' row 64 (ones column) = softmax denom l.
     y^T = Y'[0:64] * (1/l) via DVE recip + rank-1 broadcast matmul; r=0
     heads write yf directly, r=1 heads stage + SBUF DMA (partition shift).
  3. Projection partial[q, :] = yT.T @ w_proj[group rows] + b_proj/2 per
     chunk, then ONE pairwise ReduceScatter(add) on bf16 partials per
     1024-row half (2 collectives total: on HW each collective carries a
     large fixed cost, so fewer+bigger wins; the first overlaps chunks
     2-3's compute). RS result is DMA'd dram->dram into the bf16 output
     (host upcasts to f32). Rank index = parity, so the program stays
     SPMD-symmetric. Host concatenates the 8 cores' half-pieces.
"""
import numpy as np
import ml_dtypes

B, T, C = 4, 2048, 1024
H = 16
D = C // H  # 64
HPC = 8            # heads per core
GD = HPC * D       # 512 dims per core's head group
QC = 512           # query chunk width
NQC = T // QC      # 4 chunks
NKB = T // 128     # 16 k-blocks
NCC = C // 128     # 8 contraction chunks

_CACHE = {}


def _build_nc(rs_mode="per_chunk"):
    """rs_mode: 'per_chunk' (4 RS), 'halves' (2 RS), 'skip' (timing-only,
    wrong results: copies own partial instead of reducing)."""
    import concourse.bass as bass
    import concourse.mybir as mybir
    import concourse.tile as tile
    from concourse import bacc
    from contextlib import ExitStack

    f32 = mybir.dt.float32
    bf16 = mybir.dt.bfloat16

    nc = bacc.Bacc("TRN2", target_bir_lowering=False, debug=False, num_devices=8)

    xT = nc.declare_dram_parameter("xT", [C, T], bf16, isOutput=False)
    wq = nc.declare_dram_parameter("wq", [C, GD], bf16, isOutput=False)
    wk = nc.declare_dram_parameter("wk", [C, GD], bf16, isOutput=False)
    wv = nc.declare_dram_parameter("wv", [C, GD], bf16, isOutput=False)
    wp = nc.declare_dram_parameter("wp", [GD, C], bf16, isOutput=False)
    bq = nc.declare_dram_parameter("bq", [GD], f32, isOutput=False)
    bk = nc.declare_dram_parameter("bk", [GD], f32, isOutput=False)
    bv = nc.declare_dram_parameter("bv", [GD], f32, isOutput=False)
    bp = nc.declare_dram_parameter("bp", [C], f32, isOutput=False)
    # bf16 output, host upcasts. 'ar' mode outputs both full summed halves
    # (host slices the parity rows); other modes output this core's rows.
    out_rows = T if rs_mode == "ar" else T // 2
    out = nc.declare_dram_parameter("out", [out_rows, C], bf16, isOutput=True)

    # ReduceScatter buffers (Shared addr_space is only allowed for
    # AllGather/AllReduce outputs -> used by the 'ar' mode below)
    rs_in = nc.dram_tensor("rs_in", [T, C], bf16)
    rs_out = nc.dram_tensor("rs_out", [NQC, QC // 2, C], bf16)
    if rs_mode == "ar":
        ar_out = nc.dram_tensor("ar_out", [T, C], bf16, addr_space="Shared")

    with tile.TileContext(nc) as tc, ExitStack() as S0:
        consts = S0.enter_context(tc.tile_pool(name="consts", bufs=1))
        wp_pool = S0.enter_context(tc.tile_pool(name="wp", bufs=1))
        qk_pool = S0.enter_context(tc.tile_pool(name="qk", bufs=1))
        v_pool = S0.enter_context(tc.tile_pool(name="v", bufs=1))
        yt_pool = S0.enter_context(tc.tile_pool(name="yt", bufs=2))
        xp = S0.enter_context(tc.tile_pool(name="xp", bufs=1))
        wqkv = S0.enter_context(tc.tile_pool(name="wqkv", bufs=1))
        esp = S0.enter_context(tc.tile_pool(name="esp", bufs=4))
        rsp = S0.enter_context(tc.tile_pool(name="rsp", bufs=2))
        ob_pool = S0.enter_context(tc.tile_pool(name="ob", bufs=2))
        od_pool = S0.enter_context(tc.tile_pool(name="od", bufs=2))
        # PSUM banks: psb 2 + sps 2x2 + yps 2x1 = 8
        psb = S0.enter_context(tc.tile_pool(name="psb", bufs=2, space="PSUM"))
        sps = S0.enter_context(tc.tile_pool(name="sps", bufs=2, space="PSUM"))
        yps = S0.enter_context(tc.tile_pool(name="yps", bufs=1, space="PSUM"))

        # ---- constants ----
        # mask pair [128, 2*128]: strict lower triangle (k > q) zeroed,
        # applied multiplicatively AFTER exp, one copy per head of a pair.
        mask2 = consts.tile([128, 256], bf16, tag="mask")
        nc.gpsimd.memset(mask2, 1.0)
        for half in range(2):
            nc.gpsimd.affine_select(
                out=mask2[:, 128 * half : 128 * half + 128],
                in_=mask2[:, 128 * half : 128 * half + 128],
                compare_op=mybir.AluOpType.is_ge, fill=0.0,
                base=0, pattern=[[1, 128]], channel_multiplier=-1,
            )
        ones_t = consts.tile([128, D], bf16, tag="ones")
        nc.vector.memset(ones_t, 1.0)
        bq_t = consts.tile([128, 4], f32, tag="bq")
        bk_t = consts.tile([128, 4], f32, tag="bk")
        nc.sync.dma_start(out=bq_t, in_=bq.ap().rearrange("(o p) -> p o", p=128))
        nc.sync.dma_start(out=bk_t, in_=bk.ap().rearrange("(o p) -> p o", p=128))
        bv_bc = consts.tile([128, GD], f32, tag="bvb")
        bp_bc = consts.tile([128, C], f32, tag="bpb")

        # ---- persistent tiles ----
        wp_t = [wp_pool.tile([128, C], bf16, tag=f"wp{i}", name=f"wp{i}") for i in range(4)]
        yf = [wp_pool.tile([128, T], bf16, tag=f"yf{p}", name=f"yf{p}") for p in range(4)]
        qT = [qk_pool.tile([128, T], bf16, tag=f"qT{p}", name=f"qT{p}") for p in range(4)]
        kT = [qk_pool.tile([128, T], bf16, tag=f"kT{p}", name=f"kT{p}") for p in range(4)]
        vp = [v_pool.tile([128, HPC * 65], bf16, tag=f"vp{tb}", name=f"vp{tb}") for tb in range(NKB)]
        xT_t = [xp.tile([128, T], bf16, tag=f"x{i}", name=f"x{i}") for i in range(NCC)]
        wq_t = [wqkv.tile([128, GD], bf16, tag=f"wq{i}", name=f"wqt{i}") for i in range(NCC)]
        wk_t = [wqkv.tile([128, GD], bf16, tag=f"wk{i}", name=f"wkt{i}") for i in range(NCC)]
        wv_t = [wqkv.tile([128, GD], bf16, tag=f"wv{i}", name=f"wvt{i}") for i in range(NCC)]

        # load order: everything QKV chunk 0 needs first (wq/wk + xT cols
        # 0:512), spread across the sync/scalar/vector DGE queues so
        # descriptor processing parallelizes; then wv (V pieces start in
        # chunk 0's gaps), the rest of xT, and late-needed consts/wp.
        for i in range(NCC):
            sl = slice(128 * i, 128 * i + 128)
            nc.sync.dma_start(out=wq_t[i], in_=wq.ap()[sl, :])
            nc.scalar.dma_start(out=wk_t[i], in_=wk.ap()[sl, :])
            q = nc.sync if i % 2 == 0 else nc.scalar
            q.dma_start(out=xT_t[i][:, 0:512], in_=xT.ap()[sl, 0:512])
        for i in range(NCC):
            nc.scalar.dma_start(out=wv_t[i], in_=wv.ap()[128 * i : 128 * i + 128, :])
        nc.sync.dma_start(out=bv_bc, in_=bv.ap().partition_broadcast(128))
        for i in range(NCC):
            sl = slice(128 * i, 128 * i + 128)
            q = nc.sync if i % 2 == 0 else nc.scalar
            q.dma_start(out=xT_t[i][:, 512:2048], in_=xT.ap()[sl, 512:2048])
        for i in range(4):
            nc.sync.dma_start(out=wp_t[i], in_=wp.ap()[128 * i : 128 * i + 128, :])
        nc.sync.dma_start(out=bp_bc, in_=bp.ap().partition_broadcast(128))

        def emit_qkT_piece(p, t4):
            """q and k projection for head pair p, cols 512*t4..512*t4+512."""
            for w_t, b_col, dst in (
                (wq_t, bq_t[:, p : p + 1], qT[p]),
                (wk_t, bk_t[:, p : p + 1], kT[p]),
            ):
                ps = psb.tile([128, 512], f32, tag="psqk", name="psqk")
                for cc in range(NCC):
                    nc.tensor.matmul(
                        ps,
                        w_t[cc][:, 128 * p : 128 * p + 128],
                        xT_t[cc][:, 512 * t4 : 512 * t4 + 512],
                        start=(cc == 0), stop=(cc == NCC - 1),
                    )
                nc.vector.tensor_scalar_add(
                    dst[:, 512 * t4 : 512 * t4 + 512], ps, b_col
                )

        def emit_V(tb):
            ps = psb.tile([128, GD], f32, tag="psqk", name="psv")
            for cc in range(NCC):
                nc.tensor.matmul(
                    ps,
                    xT_t[cc][:, 128 * tb : 128 * tb + 128],
                    wv_t[cc],
                    start=(cc == 0), stop=(cc == NCC - 1),
                )
            v3 = vp[tb].rearrange("p (h e) -> p h e", e=65)
            nc.vector.tensor_add(
                v3[:, :, 0:D],
                ps.rearrange("p (h e) -> p h e", e=D),
                bv_bc.rearrange("p (h e) -> p h e", e=D),
            )
            nc.vector.memset(v3[:, :, D : D + 1], 1.0)

        def emit_attn_pair(p, c, inject=None):
            """Heads 2p, 2p+1 for query chunk c (cols 512c..512c+512).

            Software-pipelined j-loop; `inject` (the previous pair's
            deferred normalize) is emitted after this pair's second S
            block, so its recip-wait is covered by real PE work.
            Returns this pair's normalize closure for the same treatment.
            """
            J = 4 * c + 4
            base = QC * c
            Y = [yps.tile([65, 512], f32, tag=f"yc{r}", name=f"yc{r}")
                 for r in range(2)]
            # diag blocks first: the loop then ENDS on full-width unmasked
            # j's, so the pair-end join is exp-only (no mask on the path),
            # and the first emitted matmul covers all 512 columns (start) as
            # does the last (stop).
            order = list(range(4 * c, J)) + list(range(0, 4 * c))
            pend = []  # (i, j, es_tile, a)

            def flush_Y(n_keep):
                while len(pend) > n_keep:
                    i, j, es, a = pend.pop(0)
                    for r in range(2):
                        nc.tensor.matmul(
                            Y[r][:, a:512],
                            vp[j][:, 65 * (2 * p + r) : 65 * (2 * p + r) + 65],
                            es[:, 512 * r + a : 512 * r + 512],
                            start=(i == 0), stop=(i == J - 1),
                            skip_group_check=True,
                        )

            for i, j in enumerate(order):
                a = max(128 * j - base, 0)
                st = sps.tile([128, 1024], f32, tag="s", name="st")
                for r in range(2):
                    rb = slice(64 * r, 64 * r + 64)
                    nc.tensor.matmul(
                        st[:, 512 * r + a : 512 * r + 512],
                        kT[p][rb, 128 * j : 128 * j + 128],
                        qT[p][rb, base + a : base + 512],
                        start=True, stop=True,
                    )
                if i == 1 and inject is not None:
                    inject()
                flush_Y(2)  # lag-2: PE stays 2 j's ahead of Y's exp dep
                es = esp.tile([128, 1024], bf16, tag="es", name="es")
                st3 = st.rearrange("p (h q) -> p h q", q=512)
                es3 = es.rearrange("p (h q) -> p h q", q=512)
                nc.scalar.activation(
                    es3[:, :, a:512], st3[:, :, a:512],
                    mybir.ActivationFunctionType.Exp,
                    bias=0.0, scale=0.125,
                )
                if 128 * j >= base:  # diagonal block
                    nc.vector.tensor_mul(
                        es3[:, :, a : a + 128],
                        es3[:, :, a : a + 128],
                        mask2.rearrange("p (h q) -> p h q", q=128),
                    )
                pend.append((i, j, es, a))
            flush_Y(0)

            # normalize closure: y^T = Y[0:64] * (1/l); r=0 writes yf
            # directly, r=1 stages at partitions 0:64 and DMA-shifts to
            # 64:128. Emitted deferred (inside the next pair's j-loop).
            def normalize(p=p, base=base, Y=Y):
                rbf = rsp.tile([65, 1024], bf16, tag="rbf", name="rbf")
                with nc.allow_low_precision(reason="softmax denom bf16 for bcast matmul"):
                    for r in range(2):
                        nc.vector.reciprocal(
                            rbf[64:65, 512 * r : 512 * r + 512], Y[r][64:65, :]
                        )
                rbc = [sps.tile([64, 512], f32, tag="s", name=f"rbc{r}")
                       for r in range(2)]
                for r in range(2):
                    nc.tensor.matmul(
                        rbc[r], ones_t[64:65, 0:64],
                        rbf[64:65, 512 * r : 512 * r + 512],
                        start=True, stop=True,
                    )
                for r in range(2):
                    rbs = rsp.tile([64, 512], f32, tag="rbs", name="rbs")
                    nc.vector.tensor_copy(rbs, rbc[r])
                    if r == 0:
                        nc.vector.tensor_mul(
                            yf[p][0:64, base : base + 512], Y[r][0:64, :], rbs
                        )
                    else:
                        yts = yt_pool.tile([64, 512], bf16, tag="yts", name="yts")
                        nc.vector.tensor_mul(yts, Y[r][0:64, :], rbs)
                        nc.sync.dma_start(
                            out=yf[p][64:128, base : base + 512], in_=yts
                        )

            return normalize

        def emit_proj_tile(qq):
            ob = ob_pool.tile([128, C], bf16, tag="ob", name="ob")
            for cc2 in range(2):
                ps = psb.tile([128, 512], f32, tag="psqk", name="psproj")
                for dd in range(4):
                    nc.tensor.matmul(
                        ps,
                        yf[dd][:, 128 * qq : 128 * qq + 128],
                        wp_t[dd][:, 512 * cc2 : 512 * cc2 + 512],
                        start=(dd == 0), stop=(dd == 3),
                    )
                nc.vector.tensor_add(
                    ob[:, 512 * cc2 : 512 * cc2 + 512],
                    ps,
                    bp_bc[:, 512 * cc2 : 512 * cc2 + 512],
                )
            nc.sync.dma_start(
                out=rs_in.ap()[128 * qq : 128 * qq + 128, :], in_=ob
            )

        def emit_proj(c):
            for qq in range(4 * c, 4 * c + 4):
                emit_proj_tile(qq)

        def emit_rs(c):
            if rs_mode == "skip":
                nc.sync.dma_start(
                    out=rs_out.ap()[c],
                    in_=rs_in.ap()[QC * c : QC * c + QC // 2, :],
                )
                return
            nc.gpsimd.collective_compute(
                "ReduceScatter",
                mybir.AluOpType.add,
                ins=[rs_in.ap()[QC * c : QC * c + QC, :]],
                outs=[rs_out.ap()[c]],
                replica_groups=[[0, 1], [2, 3], [4, 5], [6, 7]],
            )

        def emit_rs_half(m):
            # one RS per 1024-row half -> internal staging (the verifier
            # rejects collective outputs aliased to ExternalOutput), then
            # one dram->dram DMA into `out` (bf16, no conversion)
            nc.gpsimd.collective_compute(
                "ReduceScatter",
                mybir.AluOpType.add,
                ins=[rs_in.ap()[1024 * m : 1024 * m + 1024, :]],
                outs=[
                    rs_out.ap().rearrange("c q x -> (c q) x")[
                        512 * m : 512 * m + 512, :
                    ]
                ],
                replica_groups=[[0, 1], [2, 3], [4, 5], [6, 7]],
            )
            # m=0 copy on SWDGE (a blocked HWDGE descriptor would stall the
            # sync queue that proj still needs); m=1 is the tail, where the
            # faster HWDGE queue is free
            eng = nc.gpsimd if m == 0 else nc.sync
            eng.dma_start(
                out=out.ap()[512 * m : 512 * m + 512, :],
                in_=rs_out.ap().rearrange("c q x -> (c q) x")[
                    512 * m : 512 * m + 512, :
                ],
            )

        def emit_ar_half(m):
            # AllReduce with Shared output: the documented HBM->HBM fast
            # path. Both cores get the full summed half; host slices parity
            # rows, so the program stays SPMD-symmetric.
            sl = slice(1024 * m, 1024 * m + 1024)
            nc.gpsimd.collective_compute(
                "AllReduce",
                mybir.AluOpType.add,
                ins=[rs_in.ap()[sl, :]],
                outs=[ar_out.ap()[sl, :]],
                replica_groups=[[0, 1], [2, 3], [4, 5], [6, 7]],
            )
            eng = nc.gpsimd if m == 0 else nc.sync
            eng.dma_start(out=out.ap()[sl, :], in_=ar_out.ap()[sl, :])

        def emit_out(c):
            # timing-variant path (per_chunk/skip): plain dram->dram copy
            nc.gpsimd.dma_start(
                out=out.ap()[256 * c : 256 * c + 256, :], in_=rs_out.ap()[c]
            )

        # ---- emission schedule: chunk-major pipeline with QKV pieces for
        # chunk c+1 interleaved into attention chunk c's pair gaps ----
        for p in range(4):
            emit_qkT_piece(p, 0)
        for tb in range(4):
            emit_V(tb)
        for c in range(NQC):
            if c + 1 < NQC:
                gaps = [
                    [("qk", 0, c + 1), ("qk", 1, c + 1)],
                    [("qk", 2, c + 1), ("qk", 3, c + 1)],
                    [("v", 4 * c + 4), ("v", 4 * c + 5)],
                    [("v", 4 * c + 6), ("v", 4 * c + 7)],
                ]
            elif rs_mode in ("halves", "ar"):
                # last chunk: fill ACT-bound attention slack with proj(2)
                gaps = [[("proj", qq)] for qq in range(8, 12)]
            else:
                gaps = [[], [], [], []]
            pending_norm = None
            for p in range(4):
                pending_norm = emit_attn_pair(p, c, inject=pending_norm)
                for piece in gaps[p]:
                    if piece[0] == "qk":
                        emit_qkT_piece(piece[1], piece[2])
                    elif piece[0] == "proj":
                        emit_proj_tile(piece[1])
                    else:
                        emit_V(piece[1])
            pending_norm()  # last pair's normalize, before this chunk's proj
            if not (rs_mode in ("halves", "ar") and c == 2):
                emit_proj(c)
            if rs_mode == "halves":
                if c % 2 == 1:
                    emit_rs_half(c // 2)
            elif rs_mode == "ar":
                if c % 2 == 1:
                    emit_ar_half(c // 2)
            else:
                emit_rs(c)
                emit_out(c)

    nc.finalize()
    return nc


def get_nc(rs_mode="halves"):
    key = ("nc", rs_mode)
    if key not in _CACHE:
        _CACHE[key] = _build_nc(rs_mode)
    return _CACHE[key]


def build_in_maps(x, w_attn, b_attn, w_proj, b_proj):
    bf = ml_dtypes.bfloat16
    x = np.asarray(x, dtype=np.float32)
    w_attn = np.asarray(w_attn, dtype=np.float32)
    b_attn = np.asarray(b_attn, dtype=np.float32)
    w_proj = np.asarray(w_proj, dtype=np.float32)
    b_proj = np.asarray(b_proj, dtype=np.float32)

    in_maps = []
    for core in range(8):
        b, g = core // 2, core % 2
        sl = slice(GD * g, GD * g + GD)
        in_maps.append({
            "xT": np.ascontiguousarray(x[b].T).astype(bf),
            "wq": np.ascontiguousarray(w_attn[:, 0 * C :][:, sl]).astype(bf),
            "wk": np.ascontiguousarray(w_attn[:, 1 * C :][:, sl]).astype(bf),
            "wv": np.ascontiguousarray(w_attn[:, 2 * C :][:, sl]).astype(bf),
            "wp": np.ascontiguousarray(w_proj[GD * g : GD * g + GD, :]).astype(bf),
            "bq": np.ascontiguousarray(b_attn[0 * C :][sl]),
            "bk": np.ascontiguousarray(b_attn[1 * C :][sl]),
            "bv": np.ascontiguousarray(b_attn[2 * C :][sl]),
            "bp": (b_proj * 0.5).astype(np.float32),
        })

    return in_maps


def assemble_out(results):
    # half-RS: core with parity g owns q in [1024m + 512g, 1024m + 512g + 512)
    out = np.empty((B, T, C), dtype=np.float32)
    for core in range(8):
        b, g = core // 2, core % 2
        piece = results[core]["out"].astype(np.float32)  # [1024, C] bf16
        for m in range(2):
            out[b, 1024 * m + 512 * g : 1024 * m + 512 * g + 512, :] = (
                piece[512 * m : 512 * m + 512]
            )
    return out


def kernel(x, w_attn, b_attn, w_proj, b_proj):
    from concourse.bass_utils import run_bass_kernel_spmd

    nc = get_nc()
    in_maps = build_in_maps(x, w_attn, b_attn, w_proj, b_proj)
    res = run_bass_kernel_spmd(nc, in_maps, core_ids=list(range(8)))
    return assemble_out(res.results)


# revision 46
# speedup vs baseline: 2.2763x; 2.2763x over previous
"""Causal self-attention on 8 TRN2 NeuronCores.

Sharding: 4-way data parallel over batch x 2-way tensor parallel over heads.
Core c handles batch b=c//2, head group g=c%2 (heads 8g..8g+8).

Per-core device kernel (all matmuls bf16, fp32 PSUM accumulation):
  1. QKV projection from host-pretransposed xT [C, T]:
     - qT/kT produced head-dim-on-partitions ([128, T] tiles, head pairs)
     - V produced natural [T, 64/head] with an appended ones column (V')
     QKV pieces for chunk c+1 are interleaved into attention chunk c's
     pair gaps so PE fills ACT-bound attention slack.
  2. Causal attention per head pair, query-chunk-major (512-wide chunks).
     Per (pair, chunk): software-pipelined j-loop over k-blocks:
       S^T_j[k,q] for heads (2p, 2p+1) lands in ONE [128,1024] PSUM tile
       (h0 cols 0:512 = bank 0, h1 cols 512:1024 = bank 1); the two
       matmuls use 64-row PE tiles T0/T8 and run concurrently on HW.
       ONE exp on ACT per j (3D AP over both heads, scale=1/8 folded) ->
       es_j bf16; ONE diag-mask mul on DVE per diag j (3D AP, mask pair).
       Y'[65, 512] += V'_j.T @ es_j per head, emitted with a TWO-j lag so
       PE stays ahead of exp deps; diagonal k-blocks are processed FIRST
       so the loop ends on unmasked full-width blocks (short pair-end
       join). Y' row 64 (ones column) = softmax denom l.
     y^T = Y'[0:64] * (1/l) via DVE recip + rank-1 broadcast matmul; r=0
     heads write yf directly, r=1 heads stage + SBUF DMA (partition shift).
  3. Projection partial[q, :] = yT.T @ w_proj[group rows] + b_proj/2 per
     chunk, then ONE pairwise ReduceScatter(add) on bf16 partials per
     1024-row half (2 collectives total: on HW each collective carries a
     large fixed cost, so fewer+bigger wins; the first overlaps chunks
     2-3's compute). RS result is DMA'd dram->dram into the bf16 output
     (host upcasts to f32). Rank index = parity, so the program stays
     SPMD-symmetric. Host concatenates the 8 cores' half-pieces.
"""
import numpy as np
import ml_dtypes

B, T, C = 4, 2048, 1024
H = 16
D = C // H  # 64
HPC = 8            # heads per core
GD = HPC * D       # 512 dims per core's head group
QC = 512           # query chunk width
NQC = T // QC      # 4 chunks
NKB = T // 128     # 16 k-blocks
NCC = C // 128     # 8 contraction chunks

_CACHE = {}


def _build_nc(rs_mode="per_chunk"):
    """rs_mode: 'per_chunk' (4 RS), 'halves' (2 RS), 'skip' (timing-only,
    wrong results: copies own partial instead of reducing)."""
    import concourse.bass as bass
    import concourse.mybir as mybir
    import concourse.tile as tile
    from concourse import bacc
    from contextlib import ExitStack

    f32 = mybir.dt.float32
    bf16 = mybir.dt.bfloat16

    nc = bacc.Bacc("TRN2", target_bir_lowering=False, debug=False, num_devices=8)

    xT = nc.declare_dram_parameter("xT", [C, T], bf16, isOutput=False)
    wq = nc.declare_dram_parameter("wq", [C, GD], bf16, isOutput=False)
    wk = nc.declare_dram_parameter("wk", [C, GD], bf16, isOutput=False)
    wv = nc.declare_dram_parameter("wv", [C, GD], bf16, isOutput=False)
    wp = nc.declare_dram_parameter("wp", [GD, C], bf16, isOutput=False)
    bq = nc.declare_dram_parameter("bq", [GD], f32, isOutput=False)
    bk = nc.declare_dram_parameter("bk", [GD], f32, isOutput=False)
    bv = nc.declare_dram_parameter("bv", [GD], f32, isOutput=False)
    bp = nc.declare_dram_parameter("bp", [C], f32, isOutput=False)
    # bf16 output, host upcasts. 'ar' mode outputs both full summed halves
    # (host slices the parity rows); other modes output this core's rows.
    out_rows = T if rs_mode == "ar" else T // 2
    out = nc.declare_dram_parameter("out", [out_rows, C], bf16, isOutput=True)

    # ReduceScatter buffers (Shared addr_space is only allowed for
    # AllGather/AllReduce outputs -> used by the 'ar' mode below)
    rs_in = nc.dram_tensor("rs_in", [T, C], bf16)
    rs_out = nc.dram_tensor("rs_out", [NQC, QC // 2, C], bf16)
    if rs_mode == "ar":
        ar_out = nc.dram_tensor("ar_out", [T, C], bf16, addr_space="Shared")

    with tile.TileContext(nc) as tc, ExitStack() as S0:
        consts = S0.enter_context(tc.tile_pool(name="consts", bufs=1))
        wp_pool = S0.enter_context(tc.tile_pool(name="wp", bufs=1))
        qk_pool = S0.enter_context(tc.tile_pool(name="qk", bufs=1))
        v_pool = S0.enter_context(tc.tile_pool(name="v", bufs=1))
        yt_pool = S0.enter_context(tc.tile_pool(name="yt", bufs=2))
        xp = S0.enter_context(tc.tile_pool(name="xp", bufs=1))
        wqkv = S0.enter_context(tc.tile_pool(name="wqkv", bufs=1))
        esp = S0.enter_context(tc.tile_pool(name="esp", bufs=5))
        rsp = S0.enter_context(tc.tile_pool(name="rsp", bufs=2))
        ob_pool = S0.enter_context(tc.tile_pool(name="ob", bufs=2))
        od_pool = S0.enter_context(tc.tile_pool(name="od", bufs=2))
        # PSUM banks: psb 2 + sps 2x2 + yps 2x1 = 8
        psb = S0.enter_context(tc.tile_pool(name="psb", bufs=2, space="PSUM"))
        sps = S0.enter_context(tc.tile_pool(name="sps", bufs=2, space="PSUM"))
        yps = S0.enter_context(tc.tile_pool(name="yps", bufs=1, space="PSUM"))

        # ---- constants ----
        # mask pair [128, 2*128]: strict lower triangle (k > q) zeroed,
        # applied multiplicatively AFTER exp, one copy per head of a pair.
        mask2 = consts.tile([128, 256], bf16, tag="mask")
        nc.gpsimd.memset(mask2, 1.0)
        for half in range(2):
            nc.gpsimd.affine_select(
                out=mask2[:, 128 * half : 128 * half + 128],
                in_=mask2[:, 128 * half : 128 * half + 128],
                compare_op=mybir.AluOpType.is_ge, fill=0.0,
                base=0, pattern=[[1, 128]], channel_multiplier=-1,
            )
        ones_t = consts.tile([128, D], bf16, tag="ones")
        nc.vector.memset(ones_t, 1.0)
        bq_t = consts.tile([128, 4], f32, tag="bq")
        bk_t = consts.tile([128, 4], f32, tag="bk")
        nc.sync.dma_start(out=bq_t, in_=bq.ap().rearrange("(o p) -> p o", p=128))
        nc.sync.dma_start(out=bk_t, in_=bk.ap().rearrange("(o p) -> p o", p=128))
        bv_bc = consts.tile([128, GD], f32, tag="bvb")
        bp_bc = consts.tile([128, C], f32, tag="bpb")

        # ---- persistent tiles ----
        wp_t = [wp_pool.tile([128, C], bf16, tag=f"wp{i}", name=f"wp{i}") for i in range(4)]
        yf = [wp_pool.tile([128, T], bf16, tag=f"yf{p}", name=f"yf{p}") for p in range(4)]
        qT = [qk_pool.tile([128, T], bf16, tag=f"qT{p}", name=f"qT{p}") for p in range(4)]
        kT = [qk_pool.tile([128, T], bf16, tag=f"kT{p}", name=f"kT{p}") for p in range(4)]
        vp = [v_pool.tile([128, HPC * 65], bf16, tag=f"vp{tb}", name=f"vp{tb}") for tb in range(NKB)]
        xT_t = [xp.tile([128, T], bf16, tag=f"x{i}", name=f"x{i}") for i in range(NCC)]
        wq_t = [wqkv.tile([128, GD], bf16, tag=f"wq{i}", name=f"wqt{i}") for i in range(NCC)]
        wk_t = [wqkv.tile([128, GD], bf16, tag=f"wk{i}", name=f"wkt{i}") for i in range(NCC)]
        wv_t = [wqkv.tile([128, GD], bf16, tag=f"wv{i}", name=f"wvt{i}") for i in range(NCC)]

        # load order: everything QKV chunk 0 needs first (wq/wk + xT cols
        # 0:512), spread across the sync/scalar/vector DGE queues so
        # descriptor processing parallelizes; then wv (V pieces start in
        # chunk 0's gaps), the rest of xT, and late-needed consts/wp.
        for i in range(NCC):
            sl = slice(128 * i, 128 * i + 128)
            nc.sync.dma_start(out=wq_t[i], in_=wq.ap()[sl, :])
            nc.scalar.dma_start(out=wk_t[i], in_=wk.ap()[sl, :])
            # third DMA-capable queue (SWDGE) carries xT so the first QKV
            # chains aren't paced by the weight loads' queues
            nc.gpsimd.dma_start(out=xT_t[i][:, 0:512], in_=xT.ap()[sl, 0:512])
        for i in range(NCC):
            nc.scalar.dma_start(out=wv_t[i], in_=wv.ap()[128 * i : 128 * i + 128, :])
        nc.sync.dma_start(out=bv_bc, in_=bv.ap().partition_broadcast(128))
        for i in range(NCC):
            sl = slice(128 * i, 128 * i + 128)
            q = nc.sync if i % 2 == 0 else nc.scalar
            q.dma_start(out=xT_t[i][:, 512:2048], in_=xT.ap()[sl, 512:2048])
        for i in range(4):
            nc.sync.dma_start(out=wp_t[i], in_=wp.ap()[128 * i : 128 * i + 128, :])
        nc.sync.dma_start(out=bp_bc, in_=bp.ap().partition_broadcast(128))

        def emit_qkT_piece(p, t4):
            """q and k projection for head pair p, cols 512*t4..512*t4+512."""
            for w_t, b_col, dst in (
                (wq_t, bq_t[:, p : p + 1], qT[p]),
                (wk_t, bk_t[:, p : p + 1], kT[p]),
            ):
                ps = psb.tile([128, 512], f32, tag="psqk", name="psqk")
                for cc in range(NCC):
                    nc.tensor.matmul(
                        ps,
                        w_t[cc][:, 128 * p : 128 * p + 128],
                        xT_t[cc][:, 512 * t4 : 512 * t4 + 512],
                        start=(cc == 0), stop=(cc == NCC - 1),
                    )
                nc.vector.tensor_scalar_add(
                    dst[:, 512 * t4 : 512 * t4 + 512], ps, b_col
                )

        def emit_V(tb):
            ps = psb.tile([128, GD], f32, tag="psqk", name="psv")
            for cc in range(NCC):
                nc.tensor.matmul(
                    ps,
                    xT_t[cc][:, 128 * tb : 128 * tb + 128],
                    wv_t[cc],
                    start=(cc == 0), stop=(cc == NCC - 1),
                )
            v3 = vp[tb].rearrange("p (h e) -> p h e", e=65)
            nc.vector.tensor_add(
                v3[:, :, 0:D],
                ps.rearrange("p (h e) -> p h e", e=D),
                bv_bc.rearrange("p (h e) -> p h e", e=D),
            )
            nc.vector.memset(v3[:, :, D : D + 1], 1.0)

        def emit_attn_pair(p, c, inject=None):
            """Heads 2p, 2p+1 for query chunk c (cols 512c..512c+512).

            Software-pipelined j-loop; `inject` (the previous pair's
            deferred normalize) is emitted after this pair's second S
            block, so its recip-wait is covered by real PE work.
            Returns this pair's normalize closure for the same treatment.
            """
            J = 4 * c + 4
            base = QC * c
            Y = [yps.tile([65, 512], f32, tag=f"yc{r}", name=f"yc{r}")
                 for r in range(2)]
            # diag blocks first: the loop then ENDS on full-width unmasked
            # j's, so the pair-end join is exp-only (no mask on the path),
            # and the first emitted matmul covers all 512 columns (start) as
            # does the last (stop).
            order = list(range(4 * c, J)) + list(range(0, 4 * c))
            pend = []  # (i, j, es_tile, a)

            def flush_Y(n_keep):
                while len(pend) > n_keep:
                    i, j, es, a = pend.pop(0)
                    for r in range(2):
                        nc.tensor.matmul(
                            Y[r][:, a:512],
                            vp[j][:, 65 * (2 * p + r) : 65 * (2 * p + r) + 65],
                            es[:, 512 * r + a : 512 * r + 512],
                            start=(i == 0), stop=(i == J - 1),
                            skip_group_check=True,
                        )

            for i, j in enumerate(order):
                a = max(128 * j - base, 0)
                st = sps.tile([128, 1024], f32, tag="s", name="st")
                for r in range(2):
                    rb = slice(64 * r, 64 * r + 64)
                    nc.tensor.matmul(
                        st[:, 512 * r + a : 512 * r + 512],
                        kT[p][rb, 128 * j : 128 * j + 128],
                        qT[p][rb, base + a : base + 512],
                        start=True, stop=True,
                    )
                if i == 1 and inject is not None:
                    inject()
                flush_Y(3 if i < 4 else 2)  # deeper lag at loop entry
                es = esp.tile([128, 1024], bf16, tag="es", name="es")
                st3 = st.rearrange("p (h q) -> p h q", q=512)
                es3 = es.rearrange("p (h q) -> p h q", q=512)
                nc.scalar.activation(
                    es3[:, :, a:512], st3[:, :, a:512],
                    mybir.ActivationFunctionType.Exp,
                    bias=0.0, scale=0.125,
                )
                if 128 * j >= base:  # diagonal block
                    nc.vector.tensor_mul(
                        es3[:, :, a : a + 128],
                        es3[:, :, a : a + 128],
                        mask2.rearrange("p (h q) -> p h q", q=128),
                    )
                pend.append((i, j, es, a))
            flush_Y(0)

            # normalize closure: y^T = Y[0:64] * (1/l); r=0 writes yf
            # directly, r=1 stages at partitions 0:64 and DMA-shifts to
            # 64:128. Emitted deferred (inside the next pair's j-loop).
            def normalize(p=p, base=base, Y=Y):
                rbf = rsp.tile([65, 1024], bf16, tag="rbf", name="rbf")
                with nc.allow_low_precision(reason="softmax denom bf16 for bcast matmul"):
                    for r in range(2):
                        nc.vector.reciprocal(
                            rbf[64:65, 512 * r : 512 * r + 512], Y[r][64:65, :]
                        )
                rbc = [sps.tile([64, 512], f32, tag="s", name=f"rbc{r}")
                       for r in range(2)]
                for r in range(2):
                    nc.tensor.matmul(
                        rbc[r], ones_t[64:65, 0:64],
                        rbf[64:65, 512 * r : 512 * r + 512],
                        start=True, stop=True,
                    )
                for r in range(2):
                    rbs = rsp.tile([64, 512], f32, tag="rbs", name="rbs")
                    nc.vector.tensor_copy(rbs, rbc[r])
                    if r == 0:
                        nc.vector.tensor_mul(
                            yf[p][0:64, base : base + 512], Y[r][0:64, :], rbs
                        )
                    else:
                        yts = yt_pool.tile([64, 512], bf16, tag="yts", name="yts")
                        nc.vector.tensor_mul(yts, Y[r][0:64, :], rbs)
                        nc.sync.dma_start(
                            out=yf[p][64:128, base : base + 512], in_=yts
                        )

            return normalize

        def emit_proj_tile(qq):
            ob = ob_pool.tile([128, C], bf16, tag="ob", name="ob")
            for cc2 in range(2):
                ps = psb.tile([128, 512], f32, tag="psqk", name="psproj")
                for dd in range(4):
                    nc.tensor.matmul(
                        ps,
                        yf[dd][:, 128 * qq : 128 * qq + 128],
                        wp_t[dd][:, 512 * cc2 : 512 * cc2 + 512],
                        start=(dd == 0), stop=(dd == 3),
                    )
                nc.vector.tensor_add(
                    ob[:, 512 * cc2 : 512 * cc2 + 512],
                    ps,
                    bp_bc[:, 512 * cc2 : 512 * cc2 + 512],
                )
            nc.sync.dma_start(
                out=rs_in.ap()[128 * qq : 128 * qq + 128, :], in_=ob
            )

        def emit_proj(c):
            for qq in range(4 * c, 4 * c + 4):
                emit_proj_tile(qq)

        def emit_rs(c):
            if rs_mode == "skip":
                nc.sync.dma_start(
                    out=rs_out.ap()[c],
                    in_=rs_in.ap()[QC * c : QC * c + QC // 2, :],
                )
                return
            nc.gpsimd.collective_compute(
                "ReduceScatter",
                mybir.AluOpType.add,
                ins=[rs_in.ap()[QC * c : QC * c + QC, :]],
                outs=[rs_out.ap()[c]],
                replica_groups=[[0, 1], [2, 3], [4, 5], [6, 7]],
            )

        def emit_rs_half(m):
            # one RS per 1024-row half -> internal staging (the verifier
            # rejects collective outputs aliased to ExternalOutput), then
            # one dram->dram DMA into `out` (bf16, no conversion)
            nc.gpsimd.collective_compute(
                "ReduceScatter",
                mybir.AluOpType.add,
                ins=[rs_in.ap()[1024 * m : 1024 * m + 1024, :]],
                outs=[
                    rs_out.ap().rearrange("c q x -> (c q) x")[
                        512 * m : 512 * m + 512, :
                    ]
                ],
                replica_groups=[[0, 1], [2, 3], [4, 5], [6, 7]],
            )
            # m=0 copy on SWDGE (a blocked HWDGE descriptor would stall the
            # sync queue that proj still needs); m=1 is the tail, where the
            # faster HWDGE queue is free
            eng = nc.gpsimd if m == 0 else nc.sync
            eng.dma_start(
                out=out.ap()[512 * m : 512 * m + 512, :],
                in_=rs_out.ap().rearrange("c q x -> (c q) x")[
                    512 * m : 512 * m + 512, :
                ],
            )

        def emit_ar_half(m):
            # AllReduce with Shared output: the documented HBM->HBM fast
            # path. Both cores get the full summed half; host slices parity
            # rows, so the program stays SPMD-symmetric.
            sl = slice(1024 * m, 1024 * m + 1024)
            nc.gpsimd.collective_compute(
                "AllReduce",
                mybir.AluOpType.add,
                ins=[rs_in.ap()[sl, :]],
                outs=[ar_out.ap()[sl, :]],
                replica_groups=[[0, 1], [2, 3], [4, 5], [6, 7]],
            )
            eng = nc.gpsimd if m == 0 else nc.sync
            eng.dma_start(out=out.ap()[sl, :], in_=ar_out.ap()[sl, :])

        def emit_out(c):
            # timing-variant path (per_chunk/skip): plain dram->dram copy
            nc.gpsimd.dma_start(
                out=out.ap()[256 * c : 256 * c + 256, :], in_=rs_out.ap()[c]
            )

        # ---- emission schedule: chunk-major pipeline with QKV pieces for
        # chunk c+1 interleaved into attention chunk c's pair gaps ----
        for p in range(4):
            emit_qkT_piece(p, 0)
        for tb in range(4):
            emit_V(tb)
        for c in range(NQC):
            if c + 1 < NQC:
                gaps = [
                    [("qk", 0, c + 1), ("qk", 1, c + 1)],
                    [("qk", 2, c + 1), ("qk", 3, c + 1)],
                    [("v", 4 * c + 4), ("v", 4 * c + 5)],
                    [("v", 4 * c + 6), ("v", 4 * c + 7)],
                ]
            elif rs_mode in ("halves", "ar"):
                # last chunk: fill ACT-bound attention slack with proj(2)
                gaps = [[("proj", qq)] for qq in range(8, 12)]
            else:
                gaps = [[], [], [], []]
            pending_norm = None
            for p in range(4):
                pending_norm = emit_attn_pair(p, c, inject=pending_norm)
                for piece in gaps[p]:
                    if piece[0] == "qk":
                        emit_qkT_piece(piece[1], piece[2])
                    elif piece[0] == "proj":
                        emit_proj_tile(piece[1])
                    else:
                        emit_V(piece[1])
            pending_norm()  # last pair's normalize, before this chunk's proj
            if not (rs_mode in ("halves", "ar") and c == 2):
                emit_proj(c)
            if rs_mode == "halves":
                if c % 2 == 1:
                    emit_rs_half(c // 2)
            elif rs_mode == "ar":
                if c % 2 == 1:
                    emit_ar_half(c // 2)
            else:
                emit_rs(c)
                emit_out(c)

    nc.finalize()
    return nc


def get_nc(rs_mode="halves"):
    key = ("nc", rs_mode)
    if key not in _CACHE:
        _CACHE[key] = _build_nc(rs_mode)
    return _CACHE[key]


def build_in_maps(x, w_attn, b_attn, w_proj, b_proj):
    bf = ml_dtypes.bfloat16
    x = np.asarray(x, dtype=np.float32)
    w_attn = np.asarray(w_attn, dtype=np.float32)
    b_attn = np.asarray(b_attn, dtype=np.float32)
    w_proj = np.asarray(w_proj, dtype=np.float32)
    b_proj = np.asarray(b_proj, dtype=np.float32)

    in_maps = []
    for core in range(8):
        b, g = core // 2, core % 2
        sl = slice(GD * g, GD * g + GD)
        in_maps.append({
            "xT": np.ascontiguousarray(x[b].T).astype(bf),
            "wq": np.ascontiguousarray(w_attn[:, 0 * C :][:, sl]).astype(bf),
            "wk": np.ascontiguousarray(w_attn[:, 1 * C :][:, sl]).astype(bf),
            "wv": np.ascontiguousarray(w_attn[:, 2 * C :][:, sl]).astype(bf),
            "wp": np.ascontiguousarray(w_proj[GD * g : GD * g + GD, :]).astype(bf),
            "bq": np.ascontiguousarray(b_attn[0 * C :][sl]),
            "bk": np.ascontiguousarray(b_attn[1 * C :][sl]),
            "bv": np.ascontiguousarray(b_attn[2 * C :][sl]),
            "bp": (b_proj * 0.5).astype(np.float32),
        })

    return in_maps


def assemble_out(results):
    # half-RS: core with parity g owns q in [1024m + 512g, 1024m + 512g + 512)
    out = np.empty((B, T, C), dtype=np.float32)
    for core in range(8):
        b, g = core // 2, core % 2
        piece = results[core]["out"].astype(np.float32)  # [1024, C] bf16
        for m in range(2):
            out[b, 1024 * m + 512 * g : 1024 * m + 512 * g + 512, :] = (
                piece[512 * m : 512 * m + 512]
            )
    return out


def kernel(x, w_attn, b_attn, w_proj, b_proj):
    from concourse.bass_utils import run_bass_kernel_spmd

    nc = get_nc()
    in_maps = build_in_maps(x, w_attn, b_attn, w_proj, b_proj)
    res = run_bass_kernel_spmd(nc, in_maps, core_ids=list(range(8)))
    return assemble_out(res.results)


# revision 48
# speedup vs baseline: 2.3742x; 1.0430x over previous
"""Causal self-attention on 8 TRN2 NeuronCores.

Sharding: 4-way data parallel over batch x 2-way tensor parallel over heads.
Core c handles batch b=c//2, head group g=c%2 (heads 8g..8g+8).

Per-core device kernel (all matmuls bf16, fp32 PSUM accumulation):
  1. QKV projection from host-pretransposed xT [C, T]:
     - qT/kT produced head-dim-on-partitions ([128, T] tiles, head pairs)
     - V produced natural [T, 64/head] with an appended ones column (V')
     QKV pieces for chunk c+1 are interleaved into attention chunk c's
     pair gaps so PE fills ACT-bound attention slack.
  2. Causal attention per head pair, query-chunk-major (512-wide chunks).
     Per (pair, chunk): software-pipelined j-loop over k-blocks:
       S^T_j[k,q] for heads (2p, 2p+1) lands in ONE [128,1024] PSUM tile
       (h0 cols 0:512 = bank 0, h1 cols 512:1024 = bank 1); the two
       matmuls use 64-row PE tiles T0/T8 and run concurrently on HW.
       ONE exp on ACT per j (3D AP over both heads, scale=1/8 folded) ->
       es_j bf16; ONE diag-mask mul on DVE per diag j (3D AP, mask pair).
       Y'[65, 512] += V'_j.T @ es_j per head, emitted with a TWO-j lag so
       PE stays ahead of exp deps; diagonal k-blocks are processed FIRST
       so the loop ends on unmasked full-width blocks (short pair-end
       join). Y' row 64 (ones column) = softmax denom l.
     y^T = Y'[0:64] * (1/l) via DVE recip + rank-1 broadcast matmul; r=0
     heads write yf directly, r=1 heads stage + SBUF DMA (partition shift).
  3. Projection partial[q, :] = yT.T @ w_proj[group rows] + b_proj/2 per
     chunk, then ONE pairwise ReduceScatter(add) on bf16 partials per
     1024-row half (2 collectives total: on HW each collective carries a
     large fixed cost, so fewer+bigger wins; the first overlaps chunks
     2-3's compute). RS result is DMA'd dram->dram into the bf16 output
     (host upcasts to f32). Rank index = parity, so the program stays
     SPMD-symmetric. Host concatenates the 8 cores' half-pieces.
"""
import numpy as np
import ml_dtypes

B, T, C = 4, 2048, 1024
H = 16
D = C // H  # 64
HPC = 8            # heads per core
GD = HPC * D       # 512 dims per core's head group
QC = 512           # query chunk width
NQC = T // QC      # 4 chunks
NKB = T // 128     # 16 k-blocks
NCC = C // 128     # 8 contraction chunks

_CACHE = {}


def _build_nc(rs_mode="per_chunk"):
    """rs_mode: 'per_chunk' (4 RS), 'halves' (2 RS), 'skip' (timing-only,
    wrong results: copies own partial instead of reducing)."""
    import concourse.bass as bass
    import concourse.mybir as mybir
    import concourse.tile as tile
    from concourse import bacc
    from contextlib import ExitStack

    f32 = mybir.dt.float32
    bf16 = mybir.dt.bfloat16

    nc = bacc.Bacc("TRN2", target_bir_lowering=False, debug=False, num_devices=8)

    xT = nc.declare_dram_parameter("xT", [C, T], bf16, isOutput=False)
    wq = nc.declare_dram_parameter("wq", [C, GD], bf16, isOutput=False)
    wk = nc.declare_dram_parameter("wk", [C, GD], bf16, isOutput=False)
    wv = nc.declare_dram_parameter("wv", [C, GD], bf16, isOutput=False)
    wp = nc.declare_dram_parameter("wp", [GD, C], bf16, isOutput=False)
    bq = nc.declare_dram_parameter("bq", [GD], f32, isOutput=False)
    bk = nc.declare_dram_parameter("bk", [GD], f32, isOutput=False)
    bv = nc.declare_dram_parameter("bv", [GD], f32, isOutput=False)
    bp = nc.declare_dram_parameter("bp", [C], f32, isOutput=False)
    # bf16 output, host upcasts. 'ar' mode outputs both full summed halves
    # (host slices the parity rows); other modes output this core's rows.
    out_rows = T if rs_mode == "ar" else T // 2
    out = nc.declare_dram_parameter("out", [out_rows, C], bf16, isOutput=True)

    # ReduceScatter buffers (Shared addr_space is only allowed for
    # AllGather/AllReduce outputs -> used by the 'ar' mode below)
    rs_in = nc.dram_tensor("rs_in", [T, C], bf16)
    rs_out = nc.dram_tensor("rs_out", [NQC, QC // 2, C], bf16)
    if rs_mode == "ar":
        ar_out = nc.dram_tensor("ar_out", [T, C], bf16, addr_space="Shared")

    with tile.TileContext(nc) as tc, ExitStack() as S0:
        consts = S0.enter_context(tc.tile_pool(name="consts", bufs=1))
        wp_pool = S0.enter_context(tc.tile_pool(name="wp", bufs=1))
        qk_pool = S0.enter_context(tc.tile_pool(name="qk", bufs=1))
        v_pool = S0.enter_context(tc.tile_pool(name="v", bufs=1))
        yt_pool = S0.enter_context(tc.tile_pool(name="yt", bufs=2))
        xp = S0.enter_context(tc.tile_pool(name="xp", bufs=1))
        wqkv = S0.enter_context(tc.tile_pool(name="wqkv", bufs=1))
        esp = S0.enter_context(tc.tile_pool(name="esp", bufs=5))
        rsp = S0.enter_context(tc.tile_pool(name="rsp", bufs=2))
        ob_pool = S0.enter_context(tc.tile_pool(name="ob", bufs=2))
        od_pool = S0.enter_context(tc.tile_pool(name="od", bufs=2))
        # PSUM banks: psb 2 + sps 2x2 + yps 2x1 = 8
        psb = S0.enter_context(tc.tile_pool(name="psb", bufs=2, space="PSUM"))
        sps = S0.enter_context(tc.tile_pool(name="sps", bufs=2, space="PSUM"))
        yps = S0.enter_context(tc.tile_pool(name="yps", bufs=1, space="PSUM"))

        # ---- constants ----
        # mask pair [128, 2*128]: strict lower triangle (k > q) zeroed,
        # applied multiplicatively AFTER exp, one copy per head of a pair.
        mask2 = consts.tile([128, 256], bf16, tag="mask")
        nc.gpsimd.memset(mask2, 1.0)
        for half in range(2):
            nc.gpsimd.affine_select(
                out=mask2[:, 128 * half : 128 * half + 128],
                in_=mask2[:, 128 * half : 128 * half + 128],
                compare_op=mybir.AluOpType.is_ge, fill=0.0,
                base=0, pattern=[[1, 128]], channel_multiplier=-1,
            )
        ones_t = consts.tile([128, D], bf16, tag="ones")
        nc.vector.memset(ones_t, 1.0)
        bq_t = consts.tile([128, 4], f32, tag="bq")
        bk_t = consts.tile([128, 4], f32, tag="bk")
        nc.sync.dma_start(out=bq_t, in_=bq.ap().rearrange("(o p) -> p o", p=128))
        nc.sync.dma_start(out=bk_t, in_=bk.ap().rearrange("(o p) -> p o", p=128))
        bv_bc = consts.tile([128, GD], f32, tag="bvb")
        bp_bc = consts.tile([128, C], f32, tag="bpb")

        # ---- persistent tiles ----
        wp_t = [wp_pool.tile([128, C], bf16, tag=f"wp{i}", name=f"wp{i}") for i in range(4)]
        yf = [wp_pool.tile([128, T], bf16, tag=f"yf{p}", name=f"yf{p}") for p in range(4)]
        qT = [qk_pool.tile([128, T], bf16, tag=f"qT{p}", name=f"qT{p}") for p in range(4)]
        kT = [qk_pool.tile([128, T], bf16, tag=f"kT{p}", name=f"kT{p}") for p in range(4)]
        vp = [v_pool.tile([128, HPC * 65], bf16, tag=f"vp{tb}", name=f"vp{tb}") for tb in range(NKB)]
        xT_t = [xp.tile([128, T], bf16, tag=f"x{i}", name=f"x{i}") for i in range(NCC)]
        wq_t = [wqkv.tile([128, GD], bf16, tag=f"wq{i}", name=f"wqt{i}") for i in range(NCC)]
        wk_t = [wqkv.tile([128, GD], bf16, tag=f"wk{i}", name=f"wkt{i}") for i in range(NCC)]
        wv_t = [wqkv.tile([128, GD], bf16, tag=f"wv{i}", name=f"wvt{i}") for i in range(NCC)]

        # load order: everything QKV chunk 0 needs first (wq/wk + xT cols
        # 0:512), spread across the sync/scalar/vector DGE queues so
        # descriptor processing parallelizes; then wv (V pieces start in
        # chunk 0's gaps), the rest of xT, and late-needed consts/wp.
        for i in range(NCC):
            sl = slice(128 * i, 128 * i + 128)
            nc.sync.dma_start(out=wq_t[i], in_=wq.ap()[sl, :])
            nc.scalar.dma_start(out=wk_t[i], in_=wk.ap()[sl, :])
            # third DMA-capable queue (SWDGE) carries xT so the first QKV
            # chains aren't paced by the weight loads' queues
            nc.gpsimd.dma_start(out=xT_t[i][:, 0:512], in_=xT.ap()[sl, 0:512])
        for i in range(NCC):
            nc.scalar.dma_start(out=wv_t[i], in_=wv.ap()[128 * i : 128 * i + 128, :])
        nc.sync.dma_start(out=bv_bc, in_=bv.ap().partition_broadcast(128))
        for i in range(NCC):
            sl = slice(128 * i, 128 * i + 128)
            q = nc.sync if i % 2 == 0 else nc.scalar
            q.dma_start(out=xT_t[i][:, 512:2048], in_=xT.ap()[sl, 512:2048])
        for i in range(4):
            nc.sync.dma_start(out=wp_t[i], in_=wp.ap()[128 * i : 128 * i + 128, :])
        nc.sync.dma_start(out=bp_bc, in_=bp.ap().partition_broadcast(128))

        def emit_qkT_piece(p, t4):
            """q and k projection for head pair p, cols 512*t4..512*t4+512."""
            for w_t, b_col, dst in (
                (wq_t, bq_t[:, p : p + 1], qT[p]),
                (wk_t, bk_t[:, p : p + 1], kT[p]),
            ):
                ps = psb.tile([128, 512], f32, tag="psqk", name="psqk")
                for cc in range(NCC):
                    nc.tensor.matmul(
                        ps,
                        w_t[cc][:, 128 * p : 128 * p + 128],
                        xT_t[cc][:, 512 * t4 : 512 * t4 + 512],
                        start=(cc == 0), stop=(cc == NCC - 1),
                    )
                nc.vector.tensor_scalar_add(
                    dst[:, 512 * t4 : 512 * t4 + 512], ps, b_col
                )

        def emit_V(tb):
            ps = psb.tile([128, GD], f32, tag="psqk", name="psv")
            for cc in range(NCC):
                nc.tensor.matmul(
                    ps,
                    xT_t[cc][:, 128 * tb : 128 * tb + 128],
                    wv_t[cc],
                    start=(cc == 0), stop=(cc == NCC - 1),
                )
            v3 = vp[tb].rearrange("p (h e) -> p h e", e=65)
            nc.vector.tensor_add(
                v3[:, :, 0:D],
                ps.rearrange("p (h e) -> p h e", e=D),
                bv_bc.rearrange("p (h e) -> p h e", e=D),
            )
            nc.vector.memset(v3[:, :, D : D + 1], 1.0)

        def emit_attn_pair(p, c, inject=None):
            """Heads 2p, 2p+1 for query chunk c (cols 512c..512c+512).

            Software-pipelined j-loop; `inject` (the previous pair's
            deferred normalize) is emitted after this pair's second S
            block, so its recip-wait is covered by real PE work.
            Returns this pair's normalize closure for the same treatment.
            """
            J = 4 * c + 4
            base = QC * c
            Y = [yps.tile([65, 512], f32, tag=f"yc{r}", name=f"yc{r}")
                 for r in range(2)]
            # diag blocks first: the loop then ENDS on full-width unmasked
            # j's, so the pair-end join is exp-only (no mask on the path),
            # and the first emitted matmul covers all 512 columns (start) as
            # does the last (stop).
            order = list(range(4 * c, J)) + list(range(0, 4 * c))
            pend = []  # (i, j, es_tile, a)

            def flush_Y(n_keep):
                while len(pend) > n_keep:
                    i, j, es, a = pend.pop(0)
                    for r in range(2):
                        nc.tensor.matmul(
                            Y[r][:, a:512],
                            vp[j][:, 65 * (2 * p + r) : 65 * (2 * p + r) + 65],
                            es[:, 512 * r + a : 512 * r + 512],
                            start=(i == 0), stop=(i == J - 1),
                            skip_group_check=True,
                        )

            for i, j in enumerate(order):
                a = max(128 * j - base, 0)
                st = sps.tile([128, 1024], f32, tag="s", name="st")
                for r in range(2):
                    rb = slice(64 * r, 64 * r + 64)
                    nc.tensor.matmul(
                        st[:, 512 * r + a : 512 * r + 512],
                        kT[p][rb, 128 * j : 128 * j + 128],
                        qT[p][rb, base + a : base + 512],
                        start=True, stop=True,
                    )
                if i == 1 and inject is not None:
                    inject()
                flush_Y(3 if i < 4 else 2)  # deeper lag at loop entry
                es = esp.tile([128, 1024], bf16, tag="es", name="es")
                st3 = st.rearrange("p (h q) -> p h q", q=512)
                es3 = es.rearrange("p (h q) -> p h q", q=512)
                nc.scalar.activation(
                    es3[:, :, a:512], st3[:, :, a:512],
                    mybir.ActivationFunctionType.Exp,
                    bias=0.0, scale=0.125,
                )
                if 128 * j >= base:  # diagonal block
                    nc.vector.tensor_mul(
                        es3[:, :, a : a + 128],
                        es3[:, :, a : a + 128],
                        mask2.rearrange("p (h q) -> p h q", q=128),
                    )
                pend.append((i, j, es, a))
            flush_Y(0)

            # normalize closure: y^T = Y[0:64] * (1/l); r=0 writes yf
            # directly, r=1 stages at partitions 0:64 and DMA-shifts to
            # 64:128. Emitted deferred (inside the next pair's j-loop).
            def normalize(p=p, base=base, Y=Y):
                rbf = rsp.tile([65, 1024], bf16, tag="rbf", name="rbf")
                with nc.allow_low_precision(reason="softmax denom bf16 for bcast matmul"):
                    for r in range(2):
                        nc.vector.reciprocal(
                            rbf[64:65, 512 * r : 512 * r + 512], Y[r][64:65, :]
                        )
                rbc = [sps.tile([64, 512], f32, tag="s", name=f"rbc{r}")
                       for r in range(2)]
                for r in range(2):
                    nc.tensor.matmul(
                        rbc[r], ones_t[64:65, 0:64],
                        rbf[64:65, 512 * r : 512 * r + 512],
                        start=True, stop=True,
                    )
                for r in range(2):
                    rbs = rsp.tile([64, 512], f32, tag="rbs", name="rbs")
                    nc.vector.tensor_copy(rbs, rbc[r])
                    if r == 0:
                        nc.vector.tensor_mul(
                            yf[p][0:64, base : base + 512], Y[r][0:64, :], rbs
                        )
                    else:
                        yts = yt_pool.tile([64, 512], bf16, tag="yts", name="yts")
                        nc.vector.tensor_mul(yts, Y[r][0:64, :], rbs)
                        nc.sync.dma_start(
                            out=yf[p][64:128, base : base + 512], in_=yts
                        )

            return normalize

        def emit_proj_tile(qq):
            ob = ob_pool.tile([128, C], bf16, tag="ob", name="ob")
            for cc2 in range(2):
                ps = psb.tile([128, 512], f32, tag="psqk", name="psproj")
                for dd in range(4):
                    nc.tensor.matmul(
                        ps,
                        yf[dd][:, 128 * qq : 128 * qq + 128],
                        wp_t[dd][:, 512 * cc2 : 512 * cc2 + 512],
                        start=(dd == 0), stop=(dd == 3),
                    )
                nc.vector.tensor_add(
                    ob[:, 512 * cc2 : 512 * cc2 + 512],
                    ps,
                    bp_bc[:, 512 * cc2 : 512 * cc2 + 512],
                )
            nc.sync.dma_start(
                out=rs_in.ap()[128 * qq : 128 * qq + 128, :], in_=ob
            )

        def emit_proj(c):
            for qq in range(4 * c, 4 * c + 4):
                emit_proj_tile(qq)

        def emit_rs(c):
            if rs_mode == "skip":
                nc.sync.dma_start(
                    out=rs_out.ap()[c],
                    in_=rs_in.ap()[QC * c : QC * c + QC // 2, :],
                )
                return
            nc.gpsimd.collective_compute(
                "ReduceScatter",
                mybir.AluOpType.add,
                ins=[rs_in.ap()[QC * c : QC * c + QC, :]],
                outs=[rs_out.ap()[c]],
                replica_groups=[[0, 1], [2, 3], [4, 5], [6, 7]],
            )

        def emit_rs_half(m):
            # one RS per 1024-row half -> internal staging (the verifier
            # rejects collective outputs aliased to ExternalOutput), then
            # one dram->dram DMA into `out` (bf16, no conversion)
            nc.gpsimd.collective_compute(
                "ReduceScatter",
                mybir.AluOpType.add,
                ins=[rs_in.ap()[1024 * m : 1024 * m + 1024, :]],
                outs=[
                    rs_out.ap().rearrange("c q x -> (c q) x")[
                        512 * m : 512 * m + 512, :
                    ]
                ],
                replica_groups=[[0, 1], [2, 3], [4, 5], [6, 7]],
            )
            # m=0 copy on SWDGE (a blocked HWDGE descriptor would stall the
            # sync queue that proj still needs); m=1 is the tail, where the
            # faster HWDGE queue is free
            eng = nc.gpsimd if m == 0 else nc.sync
            eng.dma_start(
                out=out.ap()[512 * m : 512 * m + 512, :],
                in_=rs_out.ap().rearrange("c q x -> (c q) x")[
                    512 * m : 512 * m + 512, :
                ],
            )

        def emit_ar_half(m):
            # AllReduce with Shared output: the documented HBM->HBM fast
            # path. Both cores get the full summed half; host slices parity
            # rows, so the program stays SPMD-symmetric.
            sl = slice(1024 * m, 1024 * m + 1024)
            nc.gpsimd.collective_compute(
                "AllReduce",
                mybir.AluOpType.add,
                ins=[rs_in.ap()[sl, :]],
                outs=[ar_out.ap()[sl, :]],
                replica_groups=[[0, 1], [2, 3], [4, 5], [6, 7]],
            )
            eng = nc.gpsimd if m == 0 else nc.sync
            eng.dma_start(out=out.ap()[sl, :], in_=ar_out.ap()[sl, :])

        def emit_out(c):
            # timing-variant path (per_chunk/skip): plain dram->dram copy
            nc.gpsimd.dma_start(
                out=out.ap()[256 * c : 256 * c + 256, :], in_=rs_out.ap()[c]
            )

        # ---- emission schedule: chunk-major pipeline with QKV pieces for
        # chunk c+1 interleaved into attention chunk c's pair gaps ----
        for p in range(4):
            emit_qkT_piece(p, 0)
        for tb in range(4):
            emit_V(tb)
        for c in range(NQC):
            if c + 1 < NQC:
                gaps = [
                    [("qk", 0, c + 1), ("qk", 1, c + 1)],
                    [("qk", 2, c + 1), ("qk", 3, c + 1)],
                    [("v", 4 * c + 4), ("v", 4 * c + 5)],
                    [("v", 4 * c + 6), ("v", 4 * c + 7)],
                ]
            elif rs_mode in ("halves", "ar"):
                # last chunk: fill ACT-bound attention slack with proj(2)
                gaps = [[("proj", qq)] for qq in range(8, 12)]
            else:
                gaps = [[], [], [], []]
            pending_norm = None
            for p in range(4):
                pending_norm = emit_attn_pair(p, c, inject=pending_norm)
                for piece in gaps[p]:
                    if piece[0] == "qk":
                        emit_qkT_piece(piece[1], piece[2])
                    elif piece[0] == "proj":
                        emit_proj_tile(piece[1])
                    else:
                        emit_V(piece[1])
            pending_norm()  # last pair's normalize, before this chunk's proj
            if not (rs_mode in ("halves", "ar") and c == 2):
                emit_proj(c)
            if rs_mode == "halves":
                if c % 2 == 1:
                    emit_rs_half(c // 2)
            elif rs_mode == "ar":
                if c % 2 == 1:
                    emit_ar_half(c // 2)
            else:
                emit_rs(c)
                emit_out(c)

    nc.finalize()
    return nc


def get_nc(rs_mode="halves"):
    key = ("nc", rs_mode)
    if key not in _CACHE:
        _CACHE[key] = _build_nc(rs_mode)
    return _CACHE[key]


def build_in_maps(x, w_attn, b_attn, w_proj, b_proj):
    bf = ml_dtypes.bfloat16
    x = np.asarray(x, dtype=np.float32)
    w_attn = np.asarray(w_attn, dtype=np.float32)
    b_attn = np.asarray(b_attn, dtype=np.float32)
    w_proj = np.asarray(w_proj, dtype=np.float32)
    b_proj = np.asarray(b_proj, dtype=np.float32)

    in_maps = []
    for core in range(8):
        b, g = core // 2, core % 2
        sl = slice(GD * g, GD * g + GD)
        in_maps.append({
            "xT": np.ascontiguousarray(x[b].T).astype(bf),
            "wq": np.ascontiguousarray(w_attn[:, 0 * C :][:, sl]).astype(bf),
            "wk": np.ascontiguousarray(w_attn[:, 1 * C :][:, sl]).astype(bf),
            "wv": np.ascontiguousarray(w_attn[:, 2 * C :][:, sl]).astype(bf),
            "wp": np.ascontiguousarray(w_proj[GD * g : GD * g + GD, :]).astype(bf),
            "bq": np.ascontiguousarray(b_attn[0 * C :][sl]),
            "bk": np.ascontiguousarray(b_attn[1 * C :][sl]),
            "bv": np.ascontiguousarray(b_attn[2 * C :][sl]),
            "bp": (b_proj * 0.5).astype(np.float32),
        })

    return in_maps


def assemble_out(results):
    # half-RS: core with parity g owns q in [1024m + 512g, 1024m + 512g + 512)
    out = np.empty((B, T, C), dtype=np.float32)
    for core in range(8):
        b, g = core // 2, core % 2
        piece = results[core]["out"].astype(np.float32)  # [1024, C] bf16
        for m in range(2):
            out[b, 1024 * m + 512 * g : 1024 * m + 512 * g + 512, :] = (
                piece[512 * m : 512 * m + 512]
            )
    return out


def kernel(x, w_attn, b_attn, w_proj, b_proj):
    from concourse.bass_utils import run_bass_kernel_spmd

    nc = get_nc()
    in_maps = build_in_maps(x, w_attn, b_attn, w_proj, b_proj)
    res = run_bass_kernel_spmd(nc, in_maps, core_ids=list(range(8)))
    return assemble_out(res.results)
